# revision 6
# baseline (speedup 1.0000x reference)
"""Trainium2 Bass kernel for nn_AttentionBlock (B=32, C=1024, H=W=32, nh=1).

Reference computation (per batch b, with S = H*W = 1024):
    qkv = w_qkv @ x_b            # [3C, S], 1x1 conv == channel matmul
    q, k, v = split(qkv)
    logits[t,s] = (q[:,t] . k[:,s]) / sqrt(C)
    attn = softmax_s(logits)
    h[t,s] = attn[t,s] * sum_c v[c,s]
    out = w_proj @ h + b_proj + x_b

Algebraic simplifications (weight-only, precomputed on host):
  * logits = x^T (M x) with M = Wq^T Wk  -> q/k never materialized.
  * sum_c v[c,s] = (sum_c Wv[c,:]) . x[:,s] = vs.
  * softmax row-normalization is folded into the projection weights:
    out = ((Wp^T * rcp) @ e) .* vs + b + x with e = exp(scale*l - ln4).

Precision plan (fp8 e4m3 DoubleRow = 2x PE throughput, measured on HW):
  * Stage A (y = 16*M^T x): fp8 DR (as before).
  * Stage B (l = x^T y): first NB8 256-chunks of the contraction in fp8 DR
    (x8 lhsT reused from stage A's rhs; y8 quantized from psum), the rest
    in fp16.
  * Stage C (proj = wps @ e): first NC8 256-chunks in fp8 DR (wps8 =
    host-prescaled Wp^T * rcp quantized on the fly; e8 written directly by
    the exp activation with a -ln4 bias to stay under e4m3's 240 max),
    the rest in fp16.
  * 16-bit tensors use fp16 (not bf16): same PE/DVE speed, 8x less noise.
  * PSUM accumulation fp32 everywhere; the -ln4 bias self-cancels through
    the row-sum normalization; WPS=2^17 pre-scale of Wp^T keeps wps8 in
    e4m3's normal range and is folded into wvs on the host.

Sharding: data-parallel over batch, 4 batches per core on 8 cores.
"""

import os
import sys

import numpy as np

for _p in ("/opt/trn_rl_repo", "/opt/pypackages"):
    if _p not in sys.path:
        sys.path.insert(0, _p)

import ml_dtypes

import concourse.bass as bass
import concourse.tile as tile
from concourse import bacc, mybir
from concourse.bass_utils import run_bass_kernel_spmd
from concourse.tile_rust import add_dep_helper

B, C, HH, WW = 32, 1024, 32, 32
S = HH * WW          # 1024 spatial positions
P = 128              # partitions
KC = C // P          # 8 chunks along channel dim
TC = S // P          # 8 chunks along spatial (t) dim
QC = C // 256        # 4 DoubleRow chunks along contraction dim
NN = 512             # matmul moving free dim
NCH = S // NN        # 2 free-dim halves
N_CORES = 8
BPC = B // N_CORES   # batches per core
A_SCALE = 16.0       # host pre-scale of M for fp8 range
SCALE = 1.0 / (np.sqrt(float(C)) * A_SCALE)  # folded into the exp
LN4 = float(np.log(4.0))
WPS = float(2 ** 16)  # host pre-scale of Wp^T (rcp folding keeps fp8 normal)

NB8 = int(os.environ.get("KERNEL_NB8", "2"))  # stage-B fp8 256-chunks (0..4)
NC8 = int(os.environ.get("KERNEL_NC8", "4"))  # stage-C fp8 256-chunks (0..4)

f32 = mybir.dt.float32
f32r = mybir.dt.float32r
f16 = mybir.dt.float16
fp8 = mybir.dt.float8e4

N_WARMUP = int(os.environ.get("KERNEL_WARMUP", "100"))
N_FILLER = int(os.environ.get("KERNEL_FILLER", "80"))


def build_nc(bpc: int = BPC):
    nc = bacc.Bacc(
        "TRN2",
        target_bir_lowering=False,
        debug=False,
        enable_asserts=False,
    )

    # x in fp8 DoubleRow layout [q, p, i, s]: channel c = q*256 + i*128 + p
    x8_d = nc.dram_tensor("x8", [bpc, QC, P, 2, S], fp8, kind="ExternalInput")
    # x in fp16, plain chunk layout [k, p, s]: c = k*128 + p
    xbf_d = nc.dram_tensor("xbf", [bpc, KC, P, S], f16, kind="ExternalInput")
    # A16 in SBUF layout [p][mc][q][i][m]: lhsT for stage A (fp8, 16*Wk^T Wq)
    a16_d = nc.dram_tensor("a16", [P, KC, QC, 2, P], fp8, kind="ExternalInput")
    # w_proj^T * WPS stripes: [tt][p][o]
    wpt_d = nc.dram_tensor("wpt", [TC, P, C], f16, kind="ExternalInput")
    wvs_d = nc.dram_tensor("wvs", [C], f32, kind="ExternalInput")
    ones_d = nc.dram_tensor("ones", [P, P], f32r, kind="ExternalInput")
    bp_d = nc.dram_tensor("bp", [C], f32, kind="ExternalInput")
    out_d = nc.dram_tensor("out", [bpc, C, S], f32, kind="ExternalOutput")

    with tile.TileContext(nc) as tc:
        with (
            tc.tile_pool(name="weights", bufs=1) as wpool,
            tc.tile_pool(name="x8", bufs=2) as x8pool,
            tc.tile_pool(name="xbf", bufs=2) as xbfpool,
            tc.tile_pool(name="xpb", bufs=1) as xpbpool,
            tc.tile_pool(name="y", bufs=1) as ypool,
            tc.tile_pool(name="e", bufs=1) as epool,
            tc.tile_pool(name="wpts", bufs=1) as wptspool,
            tc.tile_pool(name="vacc", bufs=1) as vaccpool,
            tc.tile_pool(name="vsb", bufs=2) as vpool,
            tc.tile_pool(name="vtmp", bufs=4) as vtmppool,
            tc.tile_pool(name="osb", bufs=4) as opool,
            tc.tile_pool(name="small", bufs=40) as spool,
            tc.tile_pool(name="psA", bufs=3, space="PSUM") as psA,
            tc.tile_pool(name="psB", bufs=3, space="PSUM") as psB,
            tc.tile_pool(name="psC", bufs=2, space="PSUM") as psC,
        ):
            # ---- small resident weights first (cheap DMAs) ----
            wvs_sb = wpool.tile([P, KC], f32, tag="wvs")
            nc.sync.dma_start(wvs_sb[:], wvs_d.rearrange("(ko ki) -> ki ko", ki=P))
            bp_sb = wpool.tile([P, KC], f32, tag="bp")
            nc.sync.dma_start(bp_sb[:], bp_d.rearrange("(o p) -> p o", p=P))
            ones_sb = wpool.tile([P, P], f32r, tag="ones")
            nc.sync.dma_start(ones_sb[:], ones_d[:, :])
            # warm the PE clock (HAM) with throwaway matmuls on a memset
            # tile — no DMA dependency, so they start immediately
            wz = wpool.tile([P, P], f16, tag="wz")
            nc.vector.memset(wz[:], 0.25)
            ln4t = wpool.tile([P, 1], f32, tag="ln4")
            nc.vector.memset(ln4t[:], -LN4)
            wu = psA.tile([P, NN], f32, tag="psA")
            for _ in range(N_WARMUP):
                nc.tensor.matmul(
                    wu[:, 0:64], wz[:], wz[:, 0:64],
                    start=True, stop=True,
                )
            a16_sb = wpool.tile([P, KC, QC, 2, P], fp8, tag="a16")
            wpt_sb = wpool.tile([P, TC, C], f16, tag="wpt")
            x8_next = xbf_next = None

            for b in range(bpc):
                if b == 0:
                    x8t = x8pool.tile([P, QC, 2, S], fp8, tag="x8")
                    xbf = xbfpool.tile([P, KC, S], f16, tag="xbf")
                    # Critical startup set: A16 stripe 0 + x8 first halves —
                    # the first psum group's inputs. Everything else chains
                    # behind so concurrent DMA queues don't dilute the
                    # bandwidth the first matmuls wait on.
                    crit = [nc.sync.dma_start(a16_sb[:, 0:1], a16_d[:, 0:1])]
                    for q in range(QC):
                        crit.append(
                            nc.sync.dma_start(x8t[:, q, :, :], x8_d[b, q])
                        )
                    crit.append(nc.sync.dma_start(a16_sb[:, 1:KC], a16_d[:, 1:KC]))
                    gate = crit[-1].ins
                    noncrit = []
                    for k in range(KC):
                        noncrit.append(
                            nc.sync.dma_start(xbf[:, k, :], xbf_d[b, k])
                        )
                    for tt in range(TC):
                        noncrit.append(
                            nc.sync.dma_start(wpt_sb[:, tt, :], wpt_d[tt])
                        )
                    for inst in noncrit:
                        add_dep_helper(
                            inst.ins, gate, sync=True,
                            reason="startup: critical DMAs first",
                        )
                else:
                    # tiles + DMAs were issued during the previous batch
                    # (ahead of its output DMAs in the sync queue)
                    x8t, xbf = x8_next, xbf_next

                # ---- stage A: y = (16 M^T) x via fp8 DoubleRow ----
                # y chunks mc < 2*NB8 quantize to fp8 (DR layout) for the
                # fp8 stage-B chunks; the rest stay fp16.
                y8 = ypool.tile([P, QC, 2, S], fp8, tag="y8", name="y8") if NB8 else None
                ybf = (
                    ypool.tile([P, KC, S], f16, tag="ybf", name="ybf")
                    if NB8 < QC
                    else None
                )
                for n in range(NCH):
                    for mc in range(KC):
                        ps = psA.tile([P, NN], f32, tag="psA")
                        for q in range(QC):
                            nc.tensor.matmul(
                                ps[:],
                                a16_sb[:, mc, q, :, :],
                                x8t[:, q, :, n * NN : (n + 1) * NN],
                                start=(q == 0),
                                stop=(q == QC - 1),
                                perf_mode=mybir.MatmulPerfMode.DoubleRow,
                            )
                        if mc < 2 * NB8:
                            nc.vector.tensor_copy(
                                out=y8[:, mc // 2, mc % 2, n * NN : (n + 1) * NN],
                                in_=ps[:],
                            )
                        else:
                            nc.vector.tensor_copy(
                                out=ybf[:, mc, n * NN : (n + 1) * NN], in_=ps[:]
                            )
                        if b == 0 and n == 0 and mc == 0:
                            # keep the PE busy (HAM warm) while the remaining
                            # A16 stripes stream in
                            wuf = psA.tile([P, NN], f32, tag="psA")
                            for _ in range(N_FILLER):
                                nc.tensor.matmul(
                                    wuf[:, 0:64], wz[:], wz[:, 0:64],
                                    start=True, stop=True,
                                )

                # ---- vacc[p,s] = sum_k wvs[k*128+p] * x[k][p,s] ----
                # products on ACT (idle during stage A); tree adds on DVE.
                vacc = vaccpool.tile([P, S], f32r, tag="vacc")

                def _vprod(k):
                    vt = vtmppool.tile([P, S], f32, tag="vtmp")
                    if b == 0:
                        # b0's xbf lands late; DVE keeps ACT free for exps
                        nc.vector.tensor_scalar(
                            vt[:], xbf[:, k, :], wvs_sb[:, k : k + 1], None,
                            mybir.AluOpType.mult,
                        )
                    else:
                        nc.scalar.activation(
                            vt[:], xbf[:, k, :],
                            mybir.ActivationFunctionType.Copy,
                            scale=wvs_sb[:, k : k + 1],
                        )
                    return vt

                p0, p1 = _vprod(0), _vprod(1)
                nc.vector.tensor_tensor(p0[:], p0[:], p1[:], mybir.AluOpType.add)
                p2, p3 = _vprod(2), _vprod(3)
                nc.vector.tensor_tensor(p2[:], p2[:], p3[:], mybir.AluOpType.add)
                nc.vector.tensor_tensor(p0[:], p0[:], p2[:], mybir.AluOpType.add)
                p4, p5 = _vprod(4), _vprod(5)
                nc.vector.tensor_tensor(p4[:], p4[:], p5[:], mybir.AluOpType.add)
                p6, p7 = _vprod(6), _vprod(7)
                nc.vector.tensor_tensor(p6[:], p6[:], p7[:], mybir.AluOpType.add)
                nc.vector.tensor_tensor(p4[:], p4[:], p6[:], mybir.AluOpType.add)
                nc.vector.tensor_tensor(vacc[:], p0[:], p4[:], mybir.AluOpType.add)

                # ---- xpb = f16(x) + b_proj (residual + bias) ----
                # on DVE, draining during stage B's matmul window
                xpb = xpbpool.tile([P, KC, S], f16, tag="xpb")
                for k in range(KC):
                    nc.vector.tensor_scalar(
                        xpb[:, k, :], xbf[:, k, :], bp_sb[:, k : k + 1], None,
                        mybir.AluOpType.add,
                    )

                # ---- stage B: logits tiles, exp -> e8/ebf, row sums ----
                # tt-outer so each row-block's rcp + scaled proj weights are
                # ready long before stage C needs them
                e8 = epool.tile([P, QC, 2, S], fp8, tag="e8", name="e8") if NC8 else None
                ebf = (
                    epool.tile([P, TC, S], f16, tag="ebf", name="ebf")
                    if NC8 < QC
                    else None
                )
                wps8 = (
                    wptspool.tile([P, QC, 2, C], fp8, tag="wps8", name="wps8")
                    if NC8
                    else None
                )
                wpts = (
                    wptspool.tile([P, TC, C], f16, tag="wpts", name="wpts")
                    if NC8 < QC
                    else None
                )
                for tt in range(TC):
                    rsh = []
                    for n in range(NCH):
                        psl = psB.tile([P, NN], f32, tag="psB")
                        first = True
                        for q in range(NB8):
                            nc.tensor.matmul(
                                psl[:],
                                x8t[:, q, :, tt * P : (tt + 1) * P],
                                y8[:, q, :, n * NN : (n + 1) * NN],
                                start=first,
                                stop=(q == QC - 1),
                                perf_mode=mybir.MatmulPerfMode.DoubleRow,
                            )
                            first = False
                        for k in range(2 * NB8, KC):
                            nc.tensor.matmul(
                                psl[:],
                                xbf[:, k, tt * P : (tt + 1) * P],
                                ybf[:, k, n * NN : (n + 1) * NN],
                                start=first,
                                stop=(k == KC - 1),
                            )
                            first = False
                        rs = spool.tile([P, 1], f32, tag="rs")
                        if tt < 2 * NC8:
                            etgt = e8[:, tt // 2, tt % 2, n * NN : (n + 1) * NN]
                        else:
                            etgt = ebf[:, tt, n * NN : (n + 1) * NN]
                        nc.scalar.activation(
                            etgt, psl[:],
                            mybir.ActivationFunctionType.Exp,
                            scale=float(SCALE), bias=ln4t[:], accum_out=rs[:],
                        )
                        rsh.append(rs)
                    rst = spool.tile([P, 1], f32, tag="rst")
                    nc.vector.tensor_tensor(
                        rst[:], rsh[0][:], rsh[1][:], mybir.AluOpType.add
                    )
                    rcp = spool.tile([P, 1], f32, tag="rcp")
                    nc.vector.reciprocal(rcp[:], rst[:])
                    if tt < 2 * NC8:
                        wtgt = wps8[:, tt // 2, tt % 2, :]
                    else:
                        wtgt = wpts[:, tt, :]
                    if tt % 2 == 0:
                        nc.scalar.activation(
                            wtgt, wpt_sb[:, tt, :],
                            mybir.ActivationFunctionType.Copy,
                            scale=rcp[:],
                        )
                    else:
                        nc.vector.tensor_scalar(
                            wtgt, wpt_sb[:, tt, :], rcp[:], None,
                            mybir.AluOpType.mult,
                        )

                # ---- vs broadcast via ones-matmul (PE, 2 tiles) ----
                vsb = vpool.tile([P, S], f32, tag="vsb")
                for n in range(NCH):
                    psv = psA.tile([P, NN], f32, tag="psA")
                    nc.tensor.matmul(
                        psv[:], ones_sb[:],
                        vacc[:, n * NN : (n + 1) * NN],
                        start=True, stop=True,
                    )
                    nc.any.tensor_copy(out=vsb[:, n * NN : (n + 1) * NN], in_=psv[:])

                # ---- prefetch next batch's inputs (ahead of this batch's
                # output DMAs in the sync queue) ----
                if b + 1 < bpc:
                    x8_next = x8pool.tile([P, QC, 2, S], fp8, tag="x8")
                    xbf_next = xbfpool.tile([P, KC, S], f16, tag="xbf")
                    for q in range(QC):
                        nc.sync.dma_start(x8_next[:, q, :, :], x8_d[b + 1, q])
                    for k in range(KC):
                        nc.sync.dma_start(xbf_next[:, k, :], xbf_d[b + 1, k])

                # ---- stage C: out = (wps @ e) * vs + (x + b) ----
                cpools = (
                    [(psC, "psC"), (psA, "psA"), (psB, "psB")]
                    if b == bpc - 1
                    else [(psC, "psC")]
                )
                for oc in range(KC):
                    for n in range(NCH):
                        cp, ctag = cpools[(oc * NCH + n) % len(cpools)]
                        pso = cp.tile([P, NN], f32, tag=ctag)
                        first = True
                        for q in range(NC8):
                            nc.tensor.matmul(
                                pso[:],
                                wps8[:, q, :, oc * P : (oc + 1) * P],
                                e8[:, q, :, n * NN : (n + 1) * NN],
                                start=first,
                                stop=(q == QC - 1),
                                perf_mode=mybir.MatmulPerfMode.DoubleRow,
                            )
                            first = False
                        for tt in range(2 * NC8, TC):
                            nc.tensor.matmul(
                                pso[:],
                                wpts[:, tt, oc * P : (oc + 1) * P],
                                ebf[:, tt, n * NN : (n + 1) * NN],
                                start=first,
                                stop=(tt == TC - 1),
                            )
                            first = False
                        osb = opool.tile([P, NN], f32, tag="osb")
                        nc.vector.tensor_tensor(
                            osb[:], pso[:], vsb[:, n * NN : (n + 1) * NN],
                            mybir.AluOpType.mult,
                        )
                        nc.vector.tensor_tensor(
                            osb[:], osb[:], xpb[:, oc, n * NN : (n + 1) * NN],
                            mybir.AluOpType.add,
                        )
                        nc.sync.dma_start(
                            out_d[b, oc * P : (oc + 1) * P, n * NN : (n + 1) * NN],
                            osb[:],
                        )
    nc.compile()
    return nc


def _host_prep(w_qkv, w_proj, b_proj):
    wq = w_qkv[0:C].astype(np.float64)
    wk = w_qkv[C : 2 * C].astype(np.float64)
    wv = w_qkv[2 * C : 3 * C]
    # lhsT for y-matmul: a16[d, c] = 16*M[c, d], M = Wq^T Wk => a16 = 16*Wk^T Wq
    a16 = np.clip(A_SCALE * (wk.T @ wq), -240.0, 240.0).astype(
        ml_dtypes.float8_e4m3
    )
    # SBUF layout [p][q][i][mc][m]: contraction d = q*256 + i*128 + p,
    # output col index c = mc*128 + m
    a16_s = np.ascontiguousarray(
        a16.reshape(QC, 2, P, KC, P).transpose(2, 3, 0, 1, 4)
    )
    # fold the stage-C WPS pre-scale into the vs path
    wvs = (wv.sum(axis=0, dtype=np.float64) / WPS).astype(np.float32)
    # wpt[tt][p][o] = WPS * w_proj[o, t = tt*128 + p]
    wpt_s = np.ascontiguousarray(
        (w_proj.T * WPS).reshape(TC, P, C).astype(np.float16)
    )
    return a16_s, wpt_s, wvs, b_proj.astype(np.float32)


_NC_CACHE = {}


def _get_nc(bpc=BPC):
    if bpc not in _NC_CACHE:
        _NC_CACHE[bpc] = build_nc(bpc)
    return _NC_CACHE[bpc]


def kernel(x, w_qkv, w_proj, b_proj, _trace=False):
    x = np.asarray(x, dtype=np.float32)
    a16, wpt, wvs, bp = _host_prep(
        np.asarray(w_qkv, np.float32),
        np.asarray(w_proj, np.float32),
        np.asarray(b_proj, np.float32),
    )
    xr_full = x.reshape(B, C, S)
    # fp8 DR layout [b, q, p, i, s]: c = q*256 + i*128 + p
    x8_full = (
        np.clip(xr_full, -240.0, 240.0)
        .astype(ml_dtypes.float8_e4m3)
        .reshape(B, QC, 2, P, S)
        .transpose(0, 1, 3, 2, 4)
    )
    xbf_full = xr_full.astype(np.float16).reshape(B, KC, P, S)
    in_maps = []
    for c in range(N_CORES):
        sl = slice(c * BPC, (c + 1) * BPC)
        in_maps.append(
            {
                "x8": np.ascontiguousarray(x8_full[sl]),
                "xbf": np.ascontiguousarray(xbf_full[sl]),
                "a16": a16,
                "wpt": wpt,
                "wvs": wvs,
                "ones": np.ones((P, P), np.float32),
                "bp": bp,
            }
        )
    nc = _get_nc(BPC)
    res = run_bass_kernel_spmd(
        nc, in_maps, core_ids=list(range(N_CORES)), trace=_trace
    )
    out = np.concatenate([r["out"] for r in res.results], axis=0)
    out = out.reshape(B, C, HH, WW)
    if _trace:
        kernel.last_results = res
    return out
